# revision 51
# baseline (speedup 1.0000x reference)
"""AGNN (2-layer) distributed Bass kernel for Trainium2, 8 NeuronCores.

Strategy (graph/data parallel):
  - Node partition: core c owns nodes [8192c, 8192(c+1)); Linear weights and
    betas replicated.  Edges assigned to the core owning their dst.
  - Per layer: each core computes its local h, L2-normalizes, packs
    [nh | h] rows in bf16 (256 B rows) and AllGathers the full 65536-row
    node table into every core's HBM.
  - Edge phase per core: edges grouped into 64 chunks of 128 dst nodes.
    nh[src]/h[src] fetched per edge with gpsimd.dma_gather from the HBM
    table (graph structure is compile-time constant, so all indices and
    one-hot incidence tiles are baked on the host):
      * indices are int16, so the table is split in two 32768-row halves
        and every chunk's edges are segregated into A/B sub-lists.
      * nh[dst] per edge   = M_g one-hot matmul (PE)
      * cos                = DVE mul + free-axis reduce
      * ex = exp(beta*cos) = ACT (beta baked as immediate)
      * segment sums       = G_g = M_g.T one-hot matmuls accumulating
        [sum(ex*h) | sum(ex)] in PSUM; out = sum(ex*h)/sum(ex) per node.
  - exact edge softmax: beta*cos is bounded (|cos|<=1), so the reference's
    running-max subtraction is a mathematical no-op and is skipped.
"""

import numpy as np
import ml_dtypes

bf16 = ml_dtypes.bfloat16

# ---- problem constants (hardcoded per spec) --------------------------------
N = 65536
E = 1048576
F_IN = 128
HID = 64
NL = 2
NC = 8
NPC = N // NC              # 8192 nodes per core
CHUNK = 128                # dst nodes per chunk
NCH = NPC // CHUNK         # 64 chunks per core
TPC = NPC // CHUNK         # 64 "tiles" per core in SBUF layout
HALF_ROWS = N // 2         # int16 table split


# ===========================================================================
# Host-side graph preprocessing
# ===========================================================================
def _balance_chunks(deg):
    """Assign each core's nodes to NCH chunks of 128, balancing total
    in-degree per chunk (greedy LPT with capacity). Returns (ut, up)."""
    import heapq
    ut = np.empty(N, np.int64)
    up = np.empty(N, np.int64)
    for c in range(NC):
        nodes = np.arange(c * NPC, (c + 1) * NPC)
        order = nodes[np.argsort(-deg[nodes], kind="stable")]
        heap = [(0, t) for t in range(NCH)]
        heapq.heapify(heap)
        counts = np.zeros(NCH, np.int64)
        pending = []
        for u in order:
            tot, t = heapq.heappop(heap)
            ut[u] = t
            up[u] = counts[t]
            counts[t] += 1
            if counts[t] < CHUNK:
                heapq.heappush(heap, (tot + int(deg[u]), t))
            else:
                pending.append(None)
    return ut, up


def _prep(src, dst):
    src = np.asarray(src).astype(np.int64).ravel()
    dst = np.asarray(dst).astype(np.int64).ravel()

    deg = np.bincount(dst, minlength=N)
    ut, up = _balance_chunks(deg)
    u = np.arange(N, dtype=np.int64)
    uc = u // NPC
    # table row of node u (partition-major per core)
    table_row = uc * NPC + up * TPC + ut          # [N]

    ec = dst // NPC                               # owning core
    et = ut[dst]                                  # chunk within core
    ep = up[dst]                                  # dst partition in chunk
    erow = table_row[src]
    eh = (erow >= HALF_ROWS).astype(np.int64)     # table half

    key = (ec * NCH + et) * 2 + eh
    so = np.argsort(key, kind="stable")
    ks = key[so]
    counts = np.bincount(ks, minlength=NC * NCH * 2)
    starts = np.concatenate([[0], np.cumsum(counts)[:-1]])
    pos = np.arange(E, dtype=np.int64) - starts[ks]

    CAP = int(np.ceil(counts.max() / 128) * 128)
    NG = CAP // 128
    while (NCH * NG) % 8:
        NG += 1
    CAP = NG * 128

    sec, set_, seh, sep = ec[so], et[so], eh[so], ep[so]
    srow = (erow[so] - seh * HALF_ROWS).astype(np.int16)

    gidx = np.zeros((NC, NCH, 2, CAP), np.int16)          # pad -> row 0
    gidx[sec, set_, seh, pos] = srow

    gg, pe = pos // 128, pos % 128
    # MG[c, ch, part, 0(M), h, g, edge] / [c, ch, part, 1(G), h, g, dstp]
    fp8 = ml_dtypes.float8_e4m3
    MG = np.zeros((NC, NCH, 128, 2, 2, NG, 128), fp8)
    MG[sec, set_, sep, 0, seh, gg, pe] = 1
    MG[sec, set_, pe, 1, seh, gg, sep] = 1

    # wrapped gather indices: per (core, half) the CAP-padded slot stream of
    # all chunks, cut into 1024-idx pieces; element q of a piece ->
    # partition q%16 (replicated x8), free q//16
    npiece = NCH * NG // 8
    assert NCH * NG % 8 == 0
    stream = gidx.transpose(0, 2, 1, 3).reshape(NC, 2, npiece, 1024)
    w = stream.reshape(NC, 2, npiece, 64, 16)
    w = np.ascontiguousarray(w.transpose(0, 1, 2, 4, 3))  # [.., 16, 64]
    idxw = np.broadcast_to(w[:, :, :, None, :, :],
                           (NC, 2, npiece, 8, 16, 64))
    idxw = np.ascontiguousarray(idxw).reshape(NC, 2, npiece, 128, 64)

    return dict(CAP=CAP, NG=NG, MG=MG, idxw=idxw, table_row=table_row,
                ut=ut, up=up)


_SKIP = set()  # debug: subset of {"gather", "collective", "mm", "dve"}


# ===========================================================================
# Device kernel builder
# ===========================================================================
def _build(NG, betas):
    import concourse.bacc as bacc
    import concourse.tile as tile
    from concourse import mybir

    F32, BF16, I16 = mybir.dt.float32, mybir.dt.bfloat16, mybir.dt.int16
    CAP = NG * 128

    nc = bacc.Bacc("TRN2", target_bir_lowering=False, debug=False,
                   num_devices=NC)

    xt = nc.dram_tensor("xt", [F_IN, NPC], F32, kind="ExternalInput").ap()
    w1 = nc.dram_tensor("w1", [F_IN, HID], F32, kind="ExternalInput").ap()
    b1 = nc.dram_tensor("b1", [1, HID], F32, kind="ExternalInput").ap()
    w2 = nc.dram_tensor("w2", [HID, HID], F32, kind="ExternalInput").ap()
    b2 = nc.dram_tensor("b2", [1, HID], F32, kind="ExternalInput").ap()
    ident = nc.dram_tensor("ident", [128, 128], F32, kind="ExternalInput").ap()
    FP8 = mybir.dt.float8e4
    NPIECE = NCH * NG // 8
    mg = nc.dram_tensor("mg", [NCH, 128, 2, 2, NG, 128], FP8,
                        kind="ExternalInput").ap()
    idxw = nc.dram_tensor("idx", [2, NPIECE, 128, 64], I16,
                          kind="ExternalInput").ap()
    outp = nc.dram_tensor("out", [NPC, HID], F32, kind="ExternalOutput").ap()

    ACT = mybir.ActivationFunctionType
    ALU = mybir.AluOpType
    AX = mybir.AxisListType

    with tile.TileContext(nc) as tc:
        with tc.tile_pool(name="pers", bufs=1) as pers, \
             tc.tile_pool(name="dram", bufs=1, space="DRAM") as dram:

            h_cur = pers.tile([128, TPC, HID], F32, tag="h_cur")
            h_nxt = pers.tile([128, TPC, HID], F32, tag="h_nxt")
            tblrow = pers.tile([128, TPC, 2 * HID], BF16, tag="tblrow")
            w1_sb = pers.tile([F_IN, HID], F32, tag="w1")
            b1_sb = pers.tile([1, HID], F32, tag="b1")
            w2_sb = pers.tile([HID, HID], F32, tag="w2")
            b2_sb = pers.tile([1, HID], F32, tag="b2")
            id_sb = pers.tile([128, 128], F32, tag="ident")
            ones_sb = pers.tile([1, 128], F32, tag="ones")
            out_all = pers.tile([128, TPC, HID], F32, tag="out_all")

            tbl_loc = dram.tile([NPC, 2 * HID], BF16, tag="tbl_loc")
            tbl_fulls = [dram.tile([N, 2 * HID], BF16, tag=f"tbl_full{L}",
                                   name=f"tbl_full{L}", addr_space="Shared")
                         for L in range(NL)]

            nc.sync.dma_start(out=w1_sb[:], in_=w1[:])
            nc.sync.dma_start(out=b1_sb[:], in_=b1[:])
            nc.sync.dma_start(out=w2_sb[:], in_=w2[:])
            nc.sync.dma_start(out=b2_sb[:], in_=b2[:])
            nc.sync.dma_start(out=id_sb[:], in_=ident[:])
            nc.vector.memset(ones_sb[:], 1.0)

            # ---------------- phase 0: h = relu(X @ W1 + b1) ----------------
            with tc.tile_pool(name="p0sb", bufs=1) as p0sb, \
                 tc.tile_pool(name="p0ps", bufs=4, space="PSUM") as p0ps:
                xt_sb = p0sb.tile([F_IN, NPC], F32, tag="xt")
                for q in range(8):
                    qs = NPC // 8
                    nc.sync.dma_start(out=xt_sb[:, q * qs:(q + 1) * qs],
                                      in_=xt[:, q * qs:(q + 1) * qs])
                for t in range(TPC):
                    pm = p0ps.tile([128, HID], F32, tag="pm")
                    nc.tensor.matmul(pm[:], lhsT=xt_sb[:, t * 128:(t + 1) * 128],
                                     rhs=w1_sb[:], start=True, stop=False)
                    nc.tensor.matmul(pm[:], lhsT=ones_sb[:], rhs=b1_sb[:],
                                     start=False, stop=True)
                    nc.scalar.activation(h_cur[:, t, :], pm[:], ACT.Relu)

            # ---------------- layers ----------------
            tl_view = tbl_loc[:].rearrange("(p t) f -> p t f", p=128)
            for L in range(NL):
                beta = float(betas[L])
                tbl_full = tbl_fulls[L]
                src_h = h_cur if L % 2 == 0 else h_nxt
                dst_h = h_nxt if L % 2 == 0 else h_cur

                # normalize + pack [nh | h] rows, bf16
                with tc.tile_pool(name=f"nrm{L}", bufs=1) as nw:
                    sq = nw.tile([128, TPC, HID], F32, tag="sq")
                    ss = nw.tile([128, TPC], F32, tag="ss")
                    rns = nw.tile([128, TPC], F32, tag="rns")
                    nc.vector.tensor_tensor(sq[:], src_h[:], src_h[:],
                                            op=ALU.mult)
                    nc.vector.tensor_reduce(ss[:], sq[:], axis=AX.X,
                                            op=ALU.add)
                    nc.vector.tensor_scalar_add(ss[:], ss[:], 1e-24)
                    nrm = nw.tile([128, TPC], F32, tag="nrm")
                    nc.scalar.activation(nrm[:], ss[:], ACT.Sqrt)
                    nc.vector.reciprocal(rns[:], nrm[:])
                    for t in range(TPC):
                        nc.vector.tensor_scalar_mul(
                            tblrow[:, t, 0:HID], src_h[:, t, :],
                            rns[:, t:t + 1])
                    nc.vector.tensor_copy(tblrow[:, :, HID:2 * HID], src_h[:])
                    nc.sync.dma_start(out=tl_view, in_=tblrow[:])
                    if "collective" not in _SKIP:
                        nc.gpsimd.collective_compute(
                            "AllGather", ALU.bypass,
                            replica_groups=[list(range(NC))],
                            ins=[tbl_loc[:].opt()], outs=[tbl_full[:].opt()])

                # edge phase: per-half slot streams, gathered in 1024-idx
                # pieces (8 groups); chunk accumulators live across pieces.
                with tc.tile_pool(name=f"ebuf{L}", bufs=12) as ebuf, \
                     tc.tile_pool(name=f"ework{L}", bufs=8) as ework, \
                     tc.tile_pool(name=f"emg{L}", bufs=4) as emg, \
                     tc.tile_pool(name=f"epsx{L}", bufs=4, space="PSUM") as epsx, \
                     tc.tile_pool(name=f"epso{L}", bufs=3, space="PSUM") as epso, \
                     tc.tile_pool(name=f"clssb{L}", bufs=3) as clssb, \
                     tc.tile_pool(name=f"clsps{L}", bufs=1, space="PSUM") as clsps:
                    mg_tiles = {}
                    po_tiles = {}

                    def get_mg(ch):
                        if ch not in mg_tiles:
                            t_ = emg.tile([128, 2, 2, NG, 128], FP8,
                                          name=f"mg{L}_{ch}", tag="mg")
                            nc.sync.dma_start(out=t_[:], in_=mg[ch])
                            mg_tiles[ch] = t_
                        return mg_tiles[ch]

                    def get_po(ch):
                        if ch not in po_tiles:
                            po_tiles[ch] = epso.tile(
                                [128, HID + 1], F32,
                                name=f"po{L}_{ch}", tag="po")
                        return po_tiles[ch]

                    for pc in range(NPIECE):
                        # prefetch mg for chunks touched by this piece and
                        # the next one, so the DMA never gates the expand mms
                        for gi_ in range(pc * 8, min((pc + 3) * 8, NCH * NG)):
                            get_mg(gi_ // NG)
                        for h_ in range(2):
                            isb = ework.tile([128, 64], I16, tag=f"idx{h_}")
                            nc.sync.dma_start(out=isb[:], in_=idxw[h_, pc])
                            b_ = ebuf.tile([128, 8, 2 * HID], BF16,
                                           tag=f"buf{h_}")
                            lo = HALF_ROWS * h_
                            if "gather" not in _SKIP:
                                nc.gpsimd.dma_gather(
                                    out_ap=b_[:], idxs_ap=isb[:],
                                    in_ap=tbl_full[lo:lo + HALF_ROWS, :],
                                    num_idxs=1024, num_idxs_reg=1024,
                                    elem_size=2 * HID)
                            else:
                                nc.vector.memset(b_[:, 0, 0:2], 0.5)
                            pe_ = epsx.tile([128, 8, HID], F32, tag="pe")
                            for j in range(8):
                                gi = pc * 8 + j
                                ch, g = gi // NG, gi % NG
                                nc.tensor.matmul(
                                    pe_[:, j, :],
                                    lhsT=get_mg(ch)[:, 0, h_, g, :],
                                    rhs=tblrow[:, ch, 0:HID],
                                    start=True, stop=True)
                            exb = ework.tile([128, 8, HID], BF16, tag="exb")
                            nc.scalar.copy(exb[:], pe_[:])
                            prod = ework.tile([128, 8, HID], BF16, tag="prod")
                            nc.vector.tensor_tensor(
                                prod[:], b_[:, :, 0:HID], exb[:], op=ALU.mult)
                            cos = ework.tile([128, 8], F32, tag="cos")
                            nc.vector.tensor_reduce(cos[:], prod[:],
                                                    axis=AX.X, op=ALU.add)
                            exh = ework.tile([128, 8], BF16, tag="exh")
                            nc.scalar.activation(exh[:], cos[:], ACT.Exp,
                                                 scale=beta)
                            hs = ework.tile([128, 8, HID + 1], BF16, tag="hs")
                            nc.vector.tensor_tensor(
                                hs[:, :, 0:HID], b_[:, :, HID:2 * HID],
                                exh[:].unsqueeze(-1).broadcast_to(
                                    [128, 8, HID]),
                                op=ALU.mult)
                            nc.vector.tensor_copy(hs[:, :, HID:HID + 1],
                                                  exh[:].unsqueeze(-1))
                            for j in range(8):
                                gi = pc * 8 + j
                                ch, g = gi // NG, gi % NG
                                nc.tensor.matmul(
                                    get_po(ch)[:],
                                    lhsT=get_mg(ch)[:, 1, h_, g, :],
                                    rhs=hs[:, j, :],
                                    start=(h_ == 0 and g == 0),
                                    stop=(h_ == 1 and g == NG - 1),
                                    skip_group_check=True)
                        # finalize chunks fully covered by pieces <= pc
                        done_hi = ((pc + 1) * 8) // NG
                        done_lo = (pc * 8) // NG
                        for ch in range(done_lo, done_hi):
                            po = po_tiles.pop(ch)
                            mg_tiles.pop(ch, None)
                            # s >= exp(-|beta|) * min_degree > 0: the graph
                            # has no zero-in-degree nodes, so 1/s is safe
                            rs = ework.tile([128, 1], F32, tag="rs")
                            nc.vector.reciprocal(rs[:], po[:, HID:HID + 1])
                            nc.scalar.activation(
                                dst_h[:, ch, :], po[:, 0:HID], ACT.Copy,
                                scale=rs[:])
                            if L == NL - 1:
                                # fused cls: out = h @ W2 + b2, per chunk so
                                # it overlaps the remaining edge phase
                                pt = clsps.tile([HID, 128], F32, tag="cls")
                                nc.tensor.transpose(pt[:], dst_h[:, ch, :],
                                                    id_sb[:])
                                ht = clssb.tile([HID, 128], F32, tag="ht")
                                nc.scalar.copy(ht[:], pt[:])
                                pm2 = clsps.tile([128, HID], F32, tag="cls")
                                nc.tensor.matmul(pm2[:], lhsT=ht[:],
                                                 rhs=w2_sb[:],
                                                 start=True, stop=False)
                                nc.tensor.matmul(pm2[:], lhsT=ones_sb[:],
                                                 rhs=b2_sb[:],
                                                 start=False, stop=True)
                                nc.scalar.copy(out_all[:, ch, :], pm2[:])

            # ---------------- final output DMA (cls fused into layer NL-1) --
            if NL > 0:
                nc.sync.dma_start(
                    out=outp[:].rearrange("(p t) f -> p t f", p=128),
                    in_=out_all[:])
            else:  # debug NL=0 path: out = relu(XW1+b1) @ W2 + b2
                with tc.tile_pool(name="fsb", bufs=4) as fsb, \
                     tc.tile_pool(name="fps", bufs=4, space="PSUM") as fps:
                    for t in range(TPC):
                        pt = fps.tile([HID, 128], F32, tag="pt")
                        nc.tensor.transpose(pt[:], h_cur[:, t, :], id_sb[:])
                        ht = fsb.tile([HID, 128], F32, tag="ht")
                        nc.scalar.copy(ht[:], pt[:])
                        pm2 = fps.tile([128, HID], F32, tag="pm2")
                        nc.tensor.matmul(pm2[:], lhsT=ht[:], rhs=w2_sb[:],
                                         start=True, stop=False)
                        nc.tensor.matmul(pm2[:], lhsT=ones_sb[:], rhs=b2_sb[:],
                                         start=False, stop=True)
                        nc.scalar.copy(out_all[:, t, :], pm2[:])
                    nc.sync.dma_start(
                        out=outp[:].rearrange("(p t) f -> p t f", p=128),
                        in_=out_all[:])
    nc.compile()
    return nc


_CACHE = {}


def _get_kernel(NG, betas):
    key = (NG, tuple(np.asarray(betas, np.float64).ravel()))
    if key not in _CACHE:
        _CACHE[key] = _build(NG, betas)
    return _CACHE[key]


# ===========================================================================
# Entry point
# ===========================================================================
def make_in_maps(inputs, pp):
    features = np.asarray(inputs["features"], np.float32)
    W1 = np.asarray(inputs["W1"], np.float32)
    b1 = np.asarray(inputs["b1"], np.float32).reshape(1, HID)
    W2 = np.asarray(inputs["W2"], np.float32)
    b2 = np.asarray(inputs["b2"], np.float32).reshape(1, HID)
    ident = np.eye(128, dtype=np.float32)
    # device-side node at (core c, partition p, tile t) is the node u with
    # (ut, up) == (t, p); phase0 lhsT column t*128+p must carry features[u]
    ut, up = pp["ut"], pp["up"]
    u = np.arange(N)
    col = (u // NPC) * NPC + ut * CHUNK + up      # phase-0 column of node u
    inv = np.empty(N, np.int64)
    inv[col] = u                                  # node at column j
    xt_full = np.ascontiguousarray(features[inv].T)   # [128, N]
    in_maps = []
    for c in range(NC):
        in_maps.append({
            "xt": np.ascontiguousarray(xt_full[:, c * NPC:(c + 1) * NPC]),
            "w1": W1, "b1": b1, "w2": W2, "b2": b2, "ident": ident,
            "mg": pp["MG"][c],
            "idx": pp["idxw"][c],
        })
    return in_maps


def unshard(results, pp):
    # output row of node u on its core: up*TPC + ut
    ut, up = pp["ut"], pp["up"]
    u = np.arange(N)
    out_cat = np.concatenate([np.asarray(results[c]["out"])
                              for c in range(NC)], axis=0)   # [N, HID]
    rows = (u // NPC) * NPC + up * TPC + ut
    return out_cat[rows]


def kernel(features, src, dst, W1, b1, W2, b2, betas):
    from concourse.bass_utils import run_bass_kernel_spmd

    betas = np.asarray(betas, np.float32)
    pp = _prep(src, dst)
    nc = _get_kernel(pp["NG"], betas)
    in_maps = make_in_maps(
        dict(features=features, W1=W1, b1=b1, W2=W2, b2=b2), pp)
    res = run_bass_kernel_spmd(nc, in_maps, core_ids=list(range(NC)))
    return unshard(res.results, pp)
